# revision 8
# baseline (speedup 1.0000x reference)
"""Causal self-attention (B=2048, T=128, C=192, H=6, D=32) on 8 TRN2 cores.

Data-parallel over batch: 256 elems/core, processed 2 at a time. v3:
engine-balanced pipeline aimed at keeping the PE at its 2.4GHz p-state.

Key changes vs v2 baseline:
  - softmax normalize: ONE broadcast tensor_tensor (stride-0 AP) instead of
    12 per-head tensor_scalar ops; mask multiply batched the same way and
    split across DVE and Pool (gpsimd).
  - exp: single batched ACTIVATE over both elems (also drains S PSUM).
  - PSUM tag-sharing (qkT->S, xT->v, PT->out) fits 8 banks with natural
    producer/consumer ordering, freeing banks for double-buffered PT.
  - PSUM drains balanced across Scalar and Vector; k-half drained on DVE
    while q-half streams so S matmuls start sooner.
"""

import sys

sys.path.insert(0, "/opt/trn_rl_repo")

import numpy as np
import ml_dtypes

N_CORES = 8
B, T, C = 2048, 128, 192
NH, HD = 6, 32
BL = B // N_CORES  # 256 per core

_CACHE = {}


def _build(bl):
    from contextlib import ExitStack

    import concourse.bass as bass
    import concourse.mybir as mybir
    import concourse.tile as tile
    from concourse import bacc

    fp32 = mybir.dt.float32
    bf16 = mybir.dt.bfloat16
    AF = mybir.ActivationFunctionType
    AX = mybir.AxisListType

    nc = bacc.Bacc("TRN2", target_bir_lowering=False, debug=False)

    x_d = nc.dram_tensor("x", [bl, T, C], fp32, kind="ExternalInput")
    wA_d = nc.dram_tensor("wA", [128, 704], bf16, kind="ExternalInput")
    wB_d = nc.dram_tensor("wB", [65, 704], bf16, kind="ExternalInput")
    wpA_d = nc.dram_tensor("wpA", [128, 192], bf16, kind="ExternalInput")
    wpB_d = nc.dram_tensor("wpB", [65, 192], bf16, kind="ExternalInput")
    tril_d = nc.dram_tensor("tril1", [128, 128], bf16, kind="ExternalInput")
    idr_d = nc.dram_tensor("identR", [128, 4, 128], bf16, kind="ExternalInput")
    out_d = nc.dram_tensor("out", [bl, T, C], fp32, kind="ExternalOutput")

    with tile.TileContext(nc) as tc, ExitStack() as ctx:
        consts = ctx.enter_context(tc.tile_pool(name="consts", bufs=1))
        sb = ctx.enter_context(tc.tile_pool(name="sb", bufs=3))
        ps = ctx.enter_context(
            tc.tile_pool(name="ps", bufs=1, space=bass.MemorySpace.PSUM)
        )

        wA = consts.tile([128, 704], bf16)
        nc.sync.dma_start(wA[:], wA_d[:])
        wB = consts.tile([65, 704], bf16)
        nc.sync.dma_start(wB[:], wB_d[:])
        wpA = consts.tile([128, 192], bf16)
        nc.sync.dma_start(wpA[:], wpA_d[:])
        wpB = consts.tile([65, 192], bf16)
        nc.sync.dma_start(wpB[:], wpB_d[:])
        tril1 = consts.tile([128, 128], bf16)
        nc.sync.dma_start(tril1[:], tril_d[:])
        identR = consts.tile([128, 4, 128], bf16)
        nc.sync.dma_start(identR[:], idr_d[:])

        for p in range(bl // 2):
            # ---- load x for 2 elems, cast to bf16 on Pool ----
            xf = sb.tile([128, 2, 192], fp32, tag="xf", bufs=4)
            nc.sync.dma_start(
                xf[:], x_d[2 * p : 2 * p + 2].rearrange("e t c -> t e c")
            )
            x16 = sb.tile([128, 2, 192], bf16, tag="x16")
            nc.gpsimd.tensor_copy(x16[:], xf[:])

            # ---- x^T via PE; drain on DVE (two pieces, junk-free) ----
            xTp = ps.tile([128, 2, 2, 128], bf16, tag="xv", name="xTp")
            for e in range(2):
                nc.tensor.transpose(
                    xTp[:, e, 0, :], x16[:, e, 0:128], identR[:, e, :]
                )
                nc.tensor.transpose(
                    xTp[0:64, e, 1, :], x16[:, e, 128:192], identR[:, 2 + e, :]
                )
            xT = sb.tile([128, 2, 2, 128], bf16, tag="xT")
            nc.vector.tensor_copy(xT[:, :, 0, :], xTp[:, :, 0, :])
            nc.vector.tensor_copy(xT[0:64, :, 1, :], xTp[0:64, :, 1, :])
            nc.gpsimd.memset(xT[64:65, :, 1, :], 1.0)

            # ---- qT/kT = W^T x^T (bias fused via ones row); k first ----
            qkTp = ps.tile([128, 4, 2, 128], fp32, tag="qs", name="qkTp")
            for j in (2, 3, 0, 1):  # k chunks first, then q
                nc.tensor.matmul(
                    qkTp[:, j, :, :],
                    wA[:, 128 * j : 128 * (j + 1)],
                    xT[:, :, 0, :],
                    start=True,
                    stop=False,
                )
                nc.tensor.matmul(
                    qkTp[:, j, :, :],
                    wB[:, 128 * j : 128 * (j + 1)],
                    xT[0:65, :, 1, :],
                    start=False,
                    stop=True,
                )
            qkT = sb.tile([128, 4, 2, 128], bf16, tag="qkT")
            # k half drained on DVE (while q still streams), q half on Scalar
            nc.vector.tensor_copy(qkT[:, 2:4, :, :], qkTp[:, 2:4, :, :])
            nc.scalar.copy(qkT[:, 0:2, :, :], qkTp[:, 0:2, :, :])

            # ---- v = x Wv (bias fused) ----
            vp = ps.tile([128, 2, 192], fp32, tag="xv", name="vp")
            for e in range(2):
                nc.tensor.matmul(
                    vp[:, e, :],
                    xT[:, e, 0, :],
                    wA[:, 512:704],
                    start=True,
                    stop=False,
                )
                nc.tensor.matmul(
                    vp[:, e, :],
                    xT[0:65, e, 1, :],
                    wB[:, 512:704],
                    start=False,
                    stop=True,
                )
            v16 = sb.tile([128, 2, 192], bf16, tag="v16")
            nc.scalar.copy(v16[:], vp[:])

            # ---- S_h = q_h^T k_h (4-head tile packing) ----
            # PSUM rule: different tile_position ROWS may not share a bank.
            # Layout: bank = h%4 (the tile row), slot i = e + 2*(h//4), so
            # each 2KB bank holds only same-row outputs. 8KB = 4 banks.
            Sp = ps.tile([128, 4, 4, 128], fp32, tag="qs", name="Sp")

            def si(e, h):
                return (h % 4, e + 2 * (h // 4))

            for e in range(2):
                for h in range(NH):
                    r = 32 * (h % 4)
                    jq = 0 if h < 4 else 1
                    br, i = si(e, h)
                    nc.tensor.matmul(
                        Sp[:, br, i, :],
                        qkT[r : r + 32, jq, e, :],
                        qkT[r : r + 32, jq + 2, e, :],
                        start=True,
                        stop=True,
                        tile_position=(r, 0),
                    )

            # valid regions: A = [0:2, 0:4] (h0,h1,h4,h5), B = [2:4, 0:2]
            # ---- P = exp(S): batched ACTIVATEs drain S ----
            P16 = sb.tile([128, 4, 4, 128], bf16, tag="P16")
            nc.scalar.activation(P16[:, 0:2, :, :], Sp[:, 0:2, :, :], AF.Exp)
            nc.scalar.activation(P16[:, 2:4, 0:2, :], Sp[:, 2:4, 0:2, :], AF.Exp)

            # ---- mask (DVE A / Pool B), rowsum, 1/r, normalize ----
            trilA = tril1[:].unsqueeze(1).unsqueeze(2).broadcast_to(
                [128, 2, 4, 128]
            )
            trilBc = tril1[:].unsqueeze(1).unsqueeze(2).broadcast_to(
                [128, 2, 2, 128]
            )
            Pm = sb.tile([128, 4, 4, 128], bf16, tag="Pm")
            nc.vector.tensor_mul(Pm[:, 0:2, :, :], P16[:, 0:2, :, :], trilA)
            nc.gpsimd.tensor_mul(
                Pm[:, 2:4, 0:2, :], P16[:, 2:4, 0:2, :], trilBc
            )
            rsum = sb.tile([128, 4, 4], fp32, tag="rsum")
            nc.vector.reduce_sum(rsum[:, 0:2, :], Pm[:, 0:2, :, :], axis=AX.X)
            nc.vector.reduce_sum(
                rsum[:, 2:4, 0:2], Pm[:, 2:4, 0:2, :], axis=AX.X
            )
            rrec = sb.tile([128, 4, 4], fp32, tag="rrec")
            nc.vector.reciprocal(rrec[:, 0:2, :], rsum[:, 0:2, :])
            nc.vector.reciprocal(rrec[:, 2:4, 0:2], rsum[:, 2:4, 0:2])
            Pn = sb.tile([128, 4, 4, 128], bf16, tag="Pn")
            rrecB = rrec[:].unsqueeze(3).broadcast_to([128, 4, 4, 128])
            nc.vector.tensor_mul(
                Pn[:, 0:2, :, :], Pm[:, 0:2, :, :], rrecB[:, 0:2, :, :]
            )
            nc.gpsimd.tensor_mul(
                Pn[:, 2:4, 0:2, :], Pm[:, 2:4, 0:2, :], rrecB[:, 2:4, 0:2, :]
            )

            # ---- P^T via PE (double-buffered PSUM); drains split V/S ----
            PT = sb.tile([128, 2, 6, 128], bf16, tag="PT")
            for e in range(2):
                PTp = ps.tile(
                    [128, 6, 128], bf16, tag="po", bufs=2, name=f"PTp_{e}"
                )
                for h in range(NH):
                    br, i = si(e, h)
                    nc.tensor.transpose(
                        PTp[:, h, :], Pn[:, br, i, :], identR[:, h % 4, :]
                    )
                if e == 0:
                    nc.scalar.copy(PT[:, e, :, :], PTp[:])
                else:
                    nc.vector.tensor_copy(PT[:, e, :, :], PTp[:])

            # ---- y^T = V^T P^T (col-tiled) ----
            yt = ps.tile([128, 2, 2, 128], fp32, tag="yt", name="yt")
            for e in range(2):
                for h in range(NH):
                    r = 32 * (h % 4)
                    j = 0 if h < 4 else 1
                    nc.tensor.matmul(
                        yt[r : r + 32, e, j, :],
                        v16[:, e, 32 * h : 32 * h + 32],
                        PT[:, e, h, :],
                        start=True,
                        stop=True,
                        tile_position=(0, r),
                    )
            yT = sb.tile([128, 2, 2, 128], bf16, tag="yT")
            nc.scalar.copy(yT[:, :, 0, :], yt[:, :, 0, :])
            nc.scalar.copy(yT[0:64, :, 1, :], yt[0:64, :, 1, :])
            nc.gpsimd.memset(yT[64:65, :, 1, :], 1.0)

            # ---- out = y W_p + b (bias via ones row) ----
            outp = ps.tile([128, 2, 192], fp32, tag="po", bufs=2, name="outp")
            for e in range(2):
                nc.tensor.matmul(
                    outp[:, e, :], yT[:, e, 0, :], wpA[:], start=True, stop=False
                )
                nc.tensor.matmul(
                    outp[:, e, :],
                    yT[0:65, e, 1, :],
                    wpB[:],
                    start=False,
                    stop=True,
                )
            outs = sb.tile([128, 2, 192], fp32, tag="outs")
            nc.scalar.copy(outs[:], outp[:])
            nc.sync.dma_start(
                out_d[2 * p : 2 * p + 2].rearrange("e t c -> t e c"), outs[:]
            )

    nc.finalize()
    return nc


def _prep_inputs(x, w_qkv, b_qkv, w_proj, b_proj, bl):
    bf = ml_dtypes.bfloat16
    scale = 1.0 / np.sqrt(HD)
    w2 = np.array(w_qkv, dtype=np.float32, copy=True)
    b2 = np.array(b_qkv, dtype=np.float32, copy=True)
    w2[:, 0:C] *= scale
    b2[0:C] *= scale
    # column order: [q h0-3 | q h4-5 + pad | k h0-3 | k h4-5 + pad | v]
    perm = np.concatenate(
        [
            np.arange(0, 128),
            np.arange(128, 192),
            np.arange(0, 64),
            np.arange(192, 320),
            np.arange(320, 384),
            np.arange(0, 64),
            np.arange(384, 576),
        ]
    )
    wA = w2[0:128][:, perm].astype(bf)
    wB = np.concatenate([w2[128:192], b2[None, :]], axis=0)[:, perm].astype(bf)
    wpA = np.asarray(w_proj)[0:128].astype(bf)
    wpB = np.concatenate(
        [np.asarray(w_proj)[128:192], np.asarray(b_proj)[None, :]], axis=0
    ).astype(bf)
    tril1 = np.tril(np.ones((128, 128), np.float32)).astype(bf)
    identR = np.broadcast_to(
        np.eye(128, dtype=np.float32), (4, 128, 128)
    ).transpose(1, 0, 2)
    identR = np.ascontiguousarray(identR).astype(bf)
    xs = np.ascontiguousarray(np.asarray(x, dtype=np.float32)).reshape(
        -1, bl, T, C
    )
    maps = []
    for i in range(xs.shape[0]):
        maps.append(
            {
                "x": xs[i],
                "wA": wA,
                "wB": wB,
                "wpA": wpA,
                "wpB": wpB,
                "tril1": tril1,
                "identR": identR,
            }
        )
    return maps


def _run(x, w_qkv, b_qkv, w_proj, b_proj, bl=BL, n_cores=N_CORES, trace=False):
    from concourse.bass_utils import run_bass_kernel_spmd

    key = bl
    if key not in _CACHE:
        _CACHE[key] = _build(bl)
    nc = _CACHE[key]
    maps = _prep_inputs(x, w_qkv, b_qkv, w_proj, b_proj, bl)[:n_cores]
    res = run_bass_kernel_spmd(
        nc, maps, core_ids=list(range(len(maps))), trace=trace
    )
    out = np.concatenate([r["out"] for r in res.results], axis=0)
    return out, res


def kernel(x, w_qkv, b_qkv, w_proj, b_proj):
    out, _ = _run(x, w_qkv, b_qkv, w_proj, b_proj)
    return out.reshape(B, T, C).astype(np.float32)


# revision 9
# speedup vs baseline: 1.1960x; 1.1960x over previous
"""Causal self-attention (B=2048, T=128, C=192, H=6, D=32) on 8 TRN2 cores.

Data-parallel over batch: 256 elems/core, 2 per iteration. v4: 2-stage
software-pipelined emission so the PE never waits on the softmax chain:
iteration i emits HEAD(i) [x load/cast/transpose, qkv matmuls+drains],
MID(i-1) [S matmuls, exp, mask/rowsum/normalize], TAIL(i-2) [P^T, y^T,
projection, store]. By the time TAIL(p)'s P^T transposes issue, softmax(p)
has had a full iteration of PE work to complete.

Engine budget per pair (measured rates): Vector = softmax core
(mask/reduce/recip/normalize A-slices) + xT drain; Scalar = exp + qkT/v/
PT-e0/yT/outs drains; Pool = x cast + B-slices + memsets; PE = 56 matmuls.

PSUM (8 banks, bank-granular slots): qs tag = qkTp(2) then Sp(4 banks,
row-bank layout: tile_position rows may not share a bank); xv = xTp/vp(1);
yt(1); po = PTp/outp double-buffered(2).
"""

import sys

sys.path.insert(0, "/opt/trn_rl_repo")

import numpy as np
import ml_dtypes

N_CORES = 8
B, T, C = 2048, 128, 192
NH, HD = 6, 32
BL = B // N_CORES  # 256 per core

_CACHE = {}


def _build(bl):
    from contextlib import ExitStack

    import concourse.bass as bass
    import concourse.mybir as mybir
    import concourse.tile as tile
    from concourse import bacc

    fp32 = mybir.dt.float32
    bf16 = mybir.dt.bfloat16
    AF = mybir.ActivationFunctionType
    AX = mybir.AxisListType

    nc = bacc.Bacc("TRN2", target_bir_lowering=False, debug=False)

    x_d = nc.dram_tensor("x", [bl, T, C], fp32, kind="ExternalInput")
    wA_d = nc.dram_tensor("wA", [128, 704], bf16, kind="ExternalInput")
    wB_d = nc.dram_tensor("wB", [65, 704], bf16, kind="ExternalInput")
    wpA_d = nc.dram_tensor("wpA", [128, 192], bf16, kind="ExternalInput")
    wpB_d = nc.dram_tensor("wpB", [65, 192], bf16, kind="ExternalInput")
    tril_d = nc.dram_tensor("tril1", [128, 128], bf16, kind="ExternalInput")
    idr_d = nc.dram_tensor("identR", [128, 4, 128], bf16, kind="ExternalInput")
    out_d = nc.dram_tensor("out", [bl, T, C], fp32, kind="ExternalOutput")

    NP = bl // 2  # pairs

    with tile.TileContext(nc) as tc, ExitStack() as ctx:
        consts = ctx.enter_context(tc.tile_pool(name="consts", bufs=1))
        sb = ctx.enter_context(tc.tile_pool(name="sb", bufs=4))
        ps = ctx.enter_context(
            tc.tile_pool(name="ps", bufs=1, space=bass.MemorySpace.PSUM)
        )

        wA = consts.tile([128, 704], bf16)
        nc.sync.dma_start(wA[:], wA_d[:])
        wB = consts.tile([65, 704], bf16)
        nc.sync.dma_start(wB[:], wB_d[:])
        wpA = consts.tile([128, 192], bf16)
        nc.sync.dma_start(wpA[:], wpA_d[:])
        wpB = consts.tile([65, 192], bf16)
        nc.sync.dma_start(wpB[:], wpB_d[:])
        tril1 = consts.tile([128, 128], bf16)
        nc.sync.dma_start(tril1[:], tril_d[:])
        identR = consts.tile([128, 4, 128], bf16)
        nc.sync.dma_start(identR[:], idr_d[:])

        # per-stage state carried across iterations
        st = {}

        def si(e, h):
            # S psum slot: bank = tile row h%4, slot = e + 2*(h//4)
            return (h % 4, e + 2 * (h // 4))

        def head(p):
            """x load, cast, transpose, qkv matmuls + drains for pair p."""
            xf = sb.tile([128, 2, 192], fp32, tag="xf", name=f"xf{p}")
            nc.sync.dma_start(
                xf[:], x_d[2 * p : 2 * p + 2].rearrange("e t c -> t e c")
            )
            x16 = sb.tile([128, 2, 192], bf16, tag="x16", name=f"x16_{p}")
            nc.gpsimd.tensor_copy(x16[:], xf[:])

            xTp = ps.tile([128, 2, 2, 128], bf16, tag="xv", name=f"xTp{p}")
            for e in range(2):
                nc.tensor.transpose(
                    xTp[:, e, 0, :], x16[:, e, 0:128], identR[:, e, :]
                )
                nc.tensor.transpose(
                    xTp[0:64, e, 1, :], x16[:, e, 128:192], identR[:, 2 + e, :]
                )
            xT = sb.tile([128, 2, 2, 128], bf16, tag="xT", name=f"xT{p}")
            nc.vector.tensor_copy(xT[:, :, 0, :], xTp[:, :, 0, :])
            nc.vector.tensor_copy(xT[0:64, :, 1, :], xTp[0:64, :, 1, :])
            nc.gpsimd.memset(xT[64:65, :, 1, :], 1.0)

            qkTp = ps.tile([128, 4, 2, 128], fp32, tag="qs", name=f"qkTp{p}")
            for j in range(4):
                nc.tensor.matmul(
                    qkTp[:, j, :, :],
                    wA[:, 128 * j : 128 * (j + 1)],
                    xT[:, :, 0, :],
                    start=True,
                    stop=False,
                )
                nc.tensor.matmul(
                    qkTp[:, j, :, :],
                    wB[:, 128 * j : 128 * (j + 1)],
                    xT[0:65, :, 1, :],
                    start=False,
                    stop=True,
                )
            qkT = sb.tile([128, 4, 2, 128], bf16, tag="qkT", name=f"qkT{p}")
            nc.scalar.copy(qkT[:], qkTp[:])

            vp = ps.tile([128, 2, 192], fp32, tag="xv", name=f"vp{p}")
            for e in range(2):
                nc.tensor.matmul(
                    vp[:, e, :],
                    xT[:, e, 0, :],
                    wA[:, 512:704],
                    start=True,
                    stop=False,
                )
                nc.tensor.matmul(
                    vp[:, e, :],
                    xT[0:65, e, 1, :],
                    wB[:, 512:704],
                    start=False,
                    stop=True,
                )
            v16 = sb.tile([128, 2, 192], bf16, tag="v16", name=f"v16_{p}")
            nc.scalar.copy(v16[:], vp[:])
            st[("qkT", p)] = qkT
            st[("v16", p)] = v16

        def mid(p):
            """S matmuls, exp, mask, rowsum, reciprocal, normalize."""
            qkT = st.pop(("qkT", p))
            Sp = ps.tile([128, 4, 4, 128], fp32, tag="qs", name=f"Sp{p}")
            for e in range(2):
                for h in range(NH):
                    r = 32 * (h % 4)
                    jq = 0 if h < 4 else 1
                    br, i = si(e, h)
                    nc.tensor.matmul(
                        Sp[:, br, i, :],
                        qkT[r : r + 32, jq, e, :],
                        qkT[r : r + 32, jq + 2, e, :],
                        start=True,
                        stop=True,
                        tile_position=(r, 0),
                    )

            # valid: A = [0:2, 0:4] (h0,h1,h4,h5), B = [2:4, 0:2] (h2,h3)
            P16 = sb.tile([128, 4, 4, 128], bf16, tag="P16", name=f"P16_{p}")
            nc.scalar.activation(P16[:, 0:2, :, :], Sp[:, 0:2, :, :], AF.Exp)
            nc.scalar.activation(
                P16[:, 2:4, 0:2, :], Sp[:, 2:4, 0:2, :], AF.Exp
            )

            trilA = tril1[:].unsqueeze(1).unsqueeze(2).broadcast_to(
                [128, 2, 4, 128]
            )
            trilB = tril1[:].unsqueeze(1).unsqueeze(2).broadcast_to(
                [128, 2, 2, 128]
            )
            Pm = sb.tile([128, 4, 4, 128], bf16, tag="Pm", name=f"Pm{p}")
            nc.vector.tensor_mul(Pm[:, 0:2, :, :], P16[:, 0:2, :, :], trilA)
            nc.gpsimd.tensor_mul(Pm[:, 2:4, 0:2, :], P16[:, 2:4, 0:2, :], trilB)
            rsum = sb.tile([128, 4, 4], fp32, tag="rsum", name=f"rs{p}")
            nc.vector.reduce_sum(rsum[:, 0:2, :], Pm[:, 0:2, :, :], axis=AX.X)
            nc.vector.reduce_sum(
                rsum[:, 2:4, 0:2], Pm[:, 2:4, 0:2, :], axis=AX.X
            )
            rrec = sb.tile([128, 4, 4], fp32, tag="rrec", name=f"rr{p}")
            nc.vector.reciprocal(rrec[:, 0:2, :], rsum[:, 0:2, :])
            nc.vector.reciprocal(rrec[:, 2:4, 0:2], rsum[:, 2:4, 0:2])
            Pn = sb.tile([128, 4, 4, 128], bf16, tag="Pn", name=f"Pn{p}")
            rrecB = rrec[:].unsqueeze(3).broadcast_to([128, 4, 4, 128])
            nc.vector.tensor_mul(
                Pn[:, 0:2, :, :], Pm[:, 0:2, :, :], rrecB[:, 0:2, :, :]
            )
            nc.gpsimd.tensor_mul(
                Pn[:, 2:4, 0:2, :], Pm[:, 2:4, 0:2, :], rrecB[:, 2:4, 0:2, :]
            )
            st[("Pn", p)] = Pn

        def tail(p):
            """P^T, y^T = V^T P^T, projection, store."""
            Pn = st.pop(("Pn", p))
            v16 = st.pop(("v16", p))
            PT = sb.tile([128, 2, 6, 128], bf16, tag="PT", name=f"PT{p}")
            for e in range(2):
                PTp = ps.tile(
                    [128, 6, 128], bf16, tag="po", bufs=2, name=f"PTp{p}_{e}"
                )
                for h in range(NH):
                    br, i = si(e, h)
                    nc.tensor.transpose(
                        PTp[:, h, :], Pn[:, br, i, :], identR[:, h % 4, :]
                    )
                if e == 0:
                    nc.scalar.copy(PT[:, e, :, :], PTp[:])
                else:
                    nc.vector.tensor_copy(PT[:, e, :, :], PTp[:])

            yt = ps.tile([128, 2, 2, 128], fp32, tag="yt", name=f"yt{p}")
            for e in range(2):
                for h in range(NH):
                    r = 32 * (h % 4)
                    j = 0 if h < 4 else 1
                    nc.tensor.matmul(
                        yt[r : r + 32, e, j, :],
                        v16[:, e, 32 * h : 32 * h + 32],
                        PT[:, e, h, :],
                        start=True,
                        stop=True,
                        tile_position=(0, r),
                    )
            yT = sb.tile([128, 2, 2, 128], bf16, tag="yT", name=f"yT{p}")
            nc.scalar.copy(yT[:, :, 0, :], yt[:, :, 0, :])
            nc.scalar.copy(yT[0:64, :, 1, :], yt[0:64, :, 1, :])
            nc.gpsimd.memset(yT[64:65, :, 1, :], 1.0)

            outp = ps.tile([128, 2, 192], fp32, tag="po", bufs=2, name=f"op{p}")
            for e in range(2):
                nc.tensor.matmul(
                    outp[:, e, :], yT[:, e, 0, :], wpA[:], start=True, stop=False
                )
                nc.tensor.matmul(
                    outp[:, e, :],
                    yT[0:65, e, 1, :],
                    wpB[:],
                    start=False,
                    stop=True,
                )
            outs = sb.tile([128, 2, 192], fp32, tag="outs", name=f"os{p}")
            nc.scalar.copy(outs[:], outp[:])
            nc.sync.dma_start(
                out_d[2 * p : 2 * p + 2].rearrange("e t c -> t e c"), outs[:]
            )

        # 2-stage skewed pipeline
        for i in range(NP + 2):
            if i < NP:
                head(i)
            if 1 <= i <= NP:
                mid(i - 1)
            if i >= 2:
                tail(i - 2)

    nc.finalize()
    return nc


def _prep_inputs(x, w_qkv, b_qkv, w_proj, b_proj, bl):
    bf = ml_dtypes.bfloat16
    scale = 1.0 / np.sqrt(HD)
    w2 = np.array(w_qkv, dtype=np.float32, copy=True)
    b2 = np.array(b_qkv, dtype=np.float32, copy=True)
    w2[:, 0:C] *= scale
    b2[0:C] *= scale
    # column order: [q h0-3 | q h4-5 + pad | k h0-3 | k h4-5 + pad | v]
    perm = np.concatenate(
        [
            np.arange(0, 128),
            np.arange(128, 192),
            np.arange(0, 64),
            np.arange(192, 320),
            np.arange(320, 384),
            np.arange(0, 64),
            np.arange(384, 576),
        ]
    )
    wA = w2[0:128][:, perm].astype(bf)
    wB = np.concatenate([w2[128:192], b2[None, :]], axis=0)[:, perm].astype(bf)
    wpA = np.asarray(w_proj)[0:128].astype(bf)
    wpB = np.concatenate(
        [np.asarray(w_proj)[128:192], np.asarray(b_proj)[None, :]], axis=0
    ).astype(bf)
    tril1 = np.tril(np.ones((128, 128), np.float32)).astype(bf)
    identR = np.broadcast_to(
        np.eye(128, dtype=np.float32), (4, 128, 128)
    ).transpose(1, 0, 2)
    identR = np.ascontiguousarray(identR).astype(bf)
    xs = np.ascontiguousarray(np.asarray(x, dtype=np.float32)).reshape(
        -1, bl, T, C
    )
    maps = []
    for i in range(xs.shape[0]):
        maps.append(
            {
                "x": xs[i],
                "wA": wA,
                "wB": wB,
                "wpA": wpA,
                "wpB": wpB,
                "tril1": tril1,
                "identR": identR,
            }
        )
    return maps


def _run(x, w_qkv, b_qkv, w_proj, b_proj, bl=BL, n_cores=N_CORES, trace=False):
    from concourse.bass_utils import run_bass_kernel_spmd

    key = bl
    if key not in _CACHE:
        _CACHE[key] = _build(bl)
    nc = _CACHE[key]
    maps = _prep_inputs(x, w_qkv, b_qkv, w_proj, b_proj, bl)[:n_cores]
    res = run_bass_kernel_spmd(
        nc, maps, core_ids=list(range(len(maps))), trace=trace
    )
    out = np.concatenate([r["out"] for r in res.results], axis=0)
    return out, res


def kernel(x, w_qkv, b_qkv, w_proj, b_proj):
    out, _ = _run(x, w_qkv, b_qkv, w_proj, b_proj)
    return out.reshape(B, T, C).astype(np.float32)


# revision 18
# speedup vs baseline: 1.4827x; 1.2398x over previous
"""Causal self-attention (B=2048, T=128, C=192, H=6, D=32) on 8 TRN2 cores.

Data-parallel over batch: 256 elems/core, 2 per iteration. v4: 2-stage
software-pipelined emission so the PE never waits on the softmax chain:
iteration i emits HEAD(i) [x load/cast/transpose, qkv matmuls+drains],
MID(i-1) [S matmuls, exp, mask/rowsum/normalize], TAIL(i-2) [P^T, y^T,
projection, store]. By the time TAIL(p)'s P^T transposes issue, softmax(p)
has had a full iteration of PE work to complete.

Engine budget per pair (measured rates): Vector = softmax core
(mask/reduce/recip/normalize A-slices) + xT drain; Scalar = exp + qkT/v/
PT-e0/yT/outs drains; Pool = x cast + B-slices + memsets; PE = 56 matmuls.

PSUM (8 banks, bank-granular slots): qs tag = qkTp(2) then Sp(4 banks,
row-bank layout: tile_position rows may not share a bank); xv = xTp/vp(1);
yt(1); po = PTp/outp double-buffered(2).
"""

import sys

sys.path.insert(0, "/opt/trn_rl_repo")

import numpy as np
import ml_dtypes

N_CORES = 8
B, T, C = 2048, 128, 192
NH, HD = 6, 32
BL = B // N_CORES  # 256 per core

_CACHE = {}


def _build(bl):
    from contextlib import ExitStack

    import concourse.bass as bass
    import concourse.mybir as mybir
    import concourse.tile as tile
    from concourse import bacc

    fp32 = mybir.dt.float32
    bf16 = mybir.dt.bfloat16
    AF = mybir.ActivationFunctionType
    AX = mybir.AxisListType

    nc = bacc.Bacc("TRN2", target_bir_lowering=False, debug=False)

    x_d = nc.dram_tensor("x", [bl, T, C], fp32, kind="ExternalInput")
    wA_d = nc.dram_tensor("wA", [128, 704], bf16, kind="ExternalInput")
    wB_d = nc.dram_tensor("wB", [65, 704], bf16, kind="ExternalInput")
    wpA_d = nc.dram_tensor("wpA", [128, 192], bf16, kind="ExternalInput")
    wpB_d = nc.dram_tensor("wpB", [65, 192], bf16, kind="ExternalInput")
    tril_d = nc.dram_tensor("tril1", [128, 128], bf16, kind="ExternalInput")
    idr_d = nc.dram_tensor("identR", [128, 4, 128], bf16, kind="ExternalInput")
    idf_d = nc.dram_tensor("identF", [128, 4, 128], fp32, kind="ExternalInput")
    out_d = nc.dram_tensor("out", [bl, T, C], fp32, kind="ExternalOutput")

    NP = bl // 2  # pairs

    with tile.TileContext(nc) as tc, ExitStack() as ctx:
        consts = ctx.enter_context(tc.tile_pool(name="consts", bufs=1))
        sb = ctx.enter_context(tc.tile_pool(name="sb", bufs=5))
        ps = ctx.enter_context(
            tc.tile_pool(name="ps", bufs=1, space=bass.MemorySpace.PSUM)
        )

        wA = consts.tile([128, 704], bf16)
        nc.sync.dma_start(wA[:], wA_d[:])
        wB = consts.tile([65, 704], bf16)
        nc.sync.dma_start(wB[:], wB_d[:])
        wpA = consts.tile([128, 192], bf16)
        nc.sync.dma_start(wpA[:], wpA_d[:])
        wpB = consts.tile([65, 192], bf16)
        nc.sync.dma_start(wpB[:], wpB_d[:])
        tril1 = consts.tile([128, 128], bf16)
        nc.sync.dma_start(tril1[:], tril_d[:])
        identR = consts.tile([128, 4, 128], bf16)
        nc.sync.dma_start(identR[:], idr_d[:])
        identF = consts.tile([128, 4, 128], fp32)
        nc.sync.dma_start(identF[:], idf_d[:])

        # per-stage state carried across iterations
        st = {}

        def si(e, h):
            # S psum slot: bank = tile row h%4, slot = e + 2*(h//4)
            return (h % 4, e + 2 * (h // 4))

        def head(p):
            """x load, fp32 transpose (the drain is the bf16 cast), qkv."""
            xf = sb.tile([128, 2, 192], fp32, tag="xf", name=f"xf{p}")
            nc.sync.dma_start(
                xf[:], x_d[2 * p : 2 * p + 2].rearrange("e t c -> t e c")
            )

            xTp = ps.tile([128, 2, 2, 128], fp32, tag="xv", name=f"xTp{p}")
            for e in range(2):
                nc.tensor.transpose(
                    xTp[:, e, 0, :], xf[:, e, 0:128], identF[:, e, :]
                )
                nc.tensor.transpose(
                    xTp[0:64, e, 1, :], xf[:, e, 128:192], identF[:, 2 + e, :]
                )
            xT = sb.tile([128, 2, 2, 128], bf16, tag="xT", name=f"xT{p}")
            nc.vector.tensor_copy(xT[:, :, 0, :], xTp[:, :, 0, :])
            nc.vector.tensor_copy(xT[0:64, :, 1, :], xTp[0:64, :, 1, :])
            nc.gpsimd.memset(xT[64:65, :, 1, :], 1.0)

            qkTp = ps.tile([128, 4, 2, 128], fp32, tag="qs", name=f"qkTp{p}")
            for j in range(4):
                nc.tensor.matmul(
                    qkTp[:, j, :, :],
                    wA[:, 128 * j : 128 * (j + 1)],
                    xT[:, :, 0, :],
                    start=True,
                    stop=False,
                )
                nc.tensor.matmul(
                    qkTp[:, j, :, :],
                    wB[:, 128 * j : 128 * (j + 1)],
                    xT[0:65, :, 1, :],
                    start=False,
                    stop=True,
                )
            qkT = sb.tile([128, 4, 2, 128], bf16, tag="qkT", name=f"qkT{p}")
            nc.scalar.copy(qkT[:], qkTp[:])

            vp = ps.tile([128, 2, 192], fp32, tag="xv", name=f"vp{p}")
            for e in range(2):
                nc.tensor.matmul(
                    vp[:, e, :],
                    xT[:, e, 0, :],
                    wA[:, 512:704],
                    start=True,
                    stop=False,
                )
                nc.tensor.matmul(
                    vp[:, e, :],
                    xT[0:65, e, 1, :],
                    wB[:, 512:704],
                    start=False,
                    stop=True,
                )
            v16 = sb.tile([128, 2, 192], bf16, tag="v16", name=f"v16_{p}")
            nc.scalar.copy(v16[:], vp[:])
            st[("qkT", p)] = qkT
            st[("v16", p)] = v16

        def mid(p):
            """S matmuls, exp, mask, rowsum, reciprocal, normalize."""
            qkT = st.pop(("qkT", p))
            Sp = ps.tile([128, 4, 4, 128], fp32, tag="qs", name=f"Sp{p}")
            for e in range(2):
                for h in range(NH):
                    r = 32 * (h % 4)
                    jq = 0 if h < 4 else 1
                    br, i = si(e, h)
                    nc.tensor.matmul(
                        Sp[:, br, i, :],
                        qkT[r : r + 32, jq, e, :],
                        qkT[r : r + 32, jq + 2, e, :],
                        start=True,
                        stop=True,
                        tile_position=(r, 0),
                    )

            # valid: A = [0:2, 0:4] (h0,h1,h4,h5), B = [2:4, 0:2] (h2,h3)
            P16 = sb.tile([128, 4, 4, 128], bf16, tag="P16", name=f"P16_{p}")
            nc.scalar.activation(P16[:, 0:2, :, :], Sp[:, 0:2, :, :], AF.Exp)
            nc.scalar.activation(
                P16[:, 2:4, 0:2, :], Sp[:, 2:4, 0:2, :], AF.Exp
            )

            trilB = tril1[:].unsqueeze(1).unsqueeze(2).broadcast_to(
                [128, 2, 2, 128]
            )
            Pm = sb.tile([128, 4, 4, 128], bf16, tag="Pm", name=f"Pm{p}")
            nc.vector.tensor_mul(
                Pm[:, 0:2, 0:2, :], P16[:, 0:2, 0:2, :], trilB
            )
            nc.gpsimd.tensor_mul(
                Pm[:, 0:2, 2:4, :], P16[:, 0:2, 2:4, :], trilB
            )
            nc.gpsimd.tensor_mul(Pm[:, 2:4, 0:2, :], P16[:, 2:4, 0:2, :], trilB)
            rsum = sb.tile([128, 4, 4], fp32, tag="rsum", name=f"rs{p}")
            nc.vector.reduce_sum(rsum[:, 0:2, :], Pm[:, 0:2, :, :], axis=AX.X)
            nc.vector.reduce_sum(
                rsum[:, 2:4, 0:2], Pm[:, 2:4, 0:2, :], axis=AX.X
            )
            rrec = sb.tile([128, 4, 4], fp32, tag="rrec", name=f"rr{p}")
            nc.vector.reciprocal(rrec[:, 0:2, :], rsum[:, 0:2, :])
            nc.vector.reciprocal(rrec[:, 2:4, 0:2], rsum[:, 2:4, 0:2])
            Pn = sb.tile([128, 4, 4, 128], bf16, tag="Pn", name=f"Pn{p}")
            rrecB = rrec[:].unsqueeze(3).broadcast_to([128, 4, 4, 128])
            nc.vector.tensor_mul(
                Pn[:, 0:2, :, :], Pm[:, 0:2, :, :], rrecB[:, 0:2, :, :]
            )
            nc.gpsimd.tensor_mul(
                Pn[:, 2:4, 0:2, :], Pm[:, 2:4, 0:2, :], rrecB[:, 2:4, 0:2, :]
            )
            st[("Pn", p)] = Pn

        def tail(p):
            """P^T, y^T = V^T P^T, projection, store."""
            Pn = st.pop(("Pn", p))
            v16 = st.pop(("v16", p))
            PT = sb.tile([128, 2, 6, 128], bf16, tag="PT", name=f"PT{p}")
            for e in range(2):
                PTp = ps.tile(
                    [128, 6, 128], bf16, tag="po", bufs=2, name=f"PTp{p}_{e}"
                )
                for h in range(NH):
                    br, i = si(e, h)
                    nc.tensor.transpose(
                        PTp[:, h, :], Pn[:, br, i, :], identR[:, h % 4, :]
                    )
                if e == 0:
                    nc.scalar.copy(PT[:, e, :, :], PTp[:])
                else:
                    nc.vector.tensor_copy(PT[:, e, :, :], PTp[:])

            yt = ps.tile([128, 2, 2, 128], fp32, tag="yt", name=f"yt{p}")
            for e in range(2):
                for h in range(NH):
                    r = 32 * (h % 4)
                    j = 0 if h < 4 else 1
                    nc.tensor.matmul(
                        yt[r : r + 32, e, j, :],
                        v16[:, e, 32 * h : 32 * h + 32],
                        PT[:, e, h, :],
                        start=True,
                        stop=True,
                        tile_position=(0, r),
                    )
            yT = sb.tile([128, 2, 2, 128], bf16, tag="yT", name=f"yT{p}")
            nc.scalar.copy(yT[:, :, 0, :], yt[:, :, 0, :])
            nc.scalar.copy(yT[0:64, :, 1, :], yt[0:64, :, 1, :])
            nc.gpsimd.memset(yT[64:65, :, 1, :], 1.0)

            outp = ps.tile([128, 2, 192], fp32, tag="po", bufs=2, name=f"op{p}")
            for e in range(2):
                nc.tensor.matmul(
                    outp[:, e, :], yT[:, e, 0, :], wpA[:], start=True, stop=False
                )
                nc.tensor.matmul(
                    outp[:, e, :],
                    yT[0:65, e, 1, :],
                    wpB[:],
                    start=False,
                    stop=True,
                )
            outs = sb.tile([128, 2, 192], fp32, tag="outs", name=f"os{p}")
            nc.scalar.copy(outs[:], outp[:])
            nc.sync.dma_start(
                out_d[2 * p : 2 * p + 2].rearrange("e t c -> t e c"), outs[:]
            )

        # 3-stage skewed pipeline: tail lags mid by 2 iterations so the
        # softmax chain has two full iterations of PE work to complete.
        for i in range(NP + 3):
            if i < NP:
                head(i)
            if 1 <= i <= NP:
                mid(i - 1)
            if i >= 3:
                tail(i - 3)

    nc.finalize()
    return nc


def _prep_inputs(x, w_qkv, b_qkv, w_proj, b_proj, bl):
    bf = ml_dtypes.bfloat16
    scale = 1.0 / np.sqrt(HD)
    w2 = np.array(w_qkv, dtype=np.float32, copy=True)
    b2 = np.array(b_qkv, dtype=np.float32, copy=True)
    w2[:, 0:C] *= scale
    b2[0:C] *= scale
    # column order: [q h0-3 | q h4-5 + pad | k h0-3 | k h4-5 + pad | v]
    perm = np.concatenate(
        [
            np.arange(0, 128),
            np.arange(128, 192),
            np.arange(0, 64),
            np.arange(192, 320),
            np.arange(320, 384),
            np.arange(0, 64),
            np.arange(384, 576),
        ]
    )
    wA = w2[0:128][:, perm].astype(bf)
    wB = np.concatenate([w2[128:192], b2[None, :]], axis=0)[:, perm].astype(bf)
    wpA = np.asarray(w_proj)[0:128].astype(bf)
    wpB = np.concatenate(
        [np.asarray(w_proj)[128:192], np.asarray(b_proj)[None, :]], axis=0
    ).astype(bf)
    tril1 = np.tril(np.ones((128, 128), np.float32)).astype(bf)
    identF = np.broadcast_to(
        np.eye(128, dtype=np.float32), (4, 128, 128)
    ).transpose(1, 0, 2)
    identF = np.ascontiguousarray(identF)
    identR = identF.astype(bf)
    xs = np.ascontiguousarray(np.asarray(x, dtype=np.float32)).reshape(
        -1, bl, T, C
    )
    maps = []
    for i in range(xs.shape[0]):
        maps.append(
            {
                "x": xs[i],
                "wA": wA,
                "wB": wB,
                "wpA": wpA,
                "wpB": wpB,
                "tril1": tril1,
                "identR": identR,
                "identF": identF,
            }
        )
    return maps


def _run(x, w_qkv, b_qkv, w_proj, b_proj, bl=BL, n_cores=N_CORES, trace=False):
    from concourse.bass_utils import run_bass_kernel_spmd

    key = bl
    if key not in _CACHE:
        _CACHE[key] = _build(bl)
    nc = _CACHE[key]
    maps = _prep_inputs(x, w_qkv, b_qkv, w_proj, b_proj, bl)[:n_cores]
    res = run_bass_kernel_spmd(
        nc, maps, core_ids=list(range(len(maps))), trace=trace
    )
    out = np.concatenate([r["out"] for r in res.results], axis=0)
    return out, res


def kernel(x, w_qkv, b_qkv, w_proj, b_proj):
    out, _ = _run(x, w_qkv, b_qkv, w_proj, b_proj)
    return out.reshape(B, T, C).astype(np.float32)
